# revision 86
# baseline (speedup 1.0000x reference)
"""Trainium2 Bass kernel for nn_Block (attention + MoE routing block).

Contract: kernel(**inputs) takes FULL unsharded inputs, returns FULL output.
Internally: two SPMD launches over 8 NeuronCores.
  Launch A: attention, tensor-parallel over heads (2 heads per core).
  Launch B: MoE, expert-parallel (1 routed expert per core) + data-parallel
            shared expert; host computes gate routing between launches.
"""

import numpy as np

# ---- problem shapes (hardcoded per contract) ----
B, S, D, H, HD = 2, 2048, 1024, 16, 64
E, TOPK = 8, 2
HM = 1024   # moe inter dim
HS = 1024   # shared expert hidden
N = B * S   # 4096 tokens
NCORES = 8
HPC = H // NCORES          # heads per core = 2
CAP = 1152                 # routed-token capacity per expert (max measured 1062)
SHARE = N // NCORES        # shared-expert tokens per core = 512
TPAD = CAP + SHARE         # 1664 = 13 * 128
EPS = 1e-6
FP32 = np.float32
KC = D // 128              # contraction blocks = 8
HC_ = HM // 128            # hidden blocks = 8
SW1, SW3, SW2 = 64.0, 16.0, 64.0   # fp8 weight scales (see _build_moe)

_CACHE = {}


# --------------------------------------------------------------------------
# device programs
# --------------------------------------------------------------------------

def _mk_bass():
    from concourse import bacc
    return bacc.Bacc(
        "TRN2",
        target_bir_lowering=False,
        debug=False,
        enable_asserts=True,
        num_devices=NCORES,
    )


def _build_attn(upto=99, rope_gpsimd=True):
    """Launch A: per-core attention for 2 heads; outputs partial (N, D) of @wo.

    bf16 data everywhere (psum accumulation fp32). Phase 3 does one merged
    [128,2048] exp per key tile so ACT runs nothing but exp.
    """
    import concourse.mybir as mybir
    import concourse.tile as tile
    from concourse.bass import ts

    dt = mybir.dt.float32
    dtr = mybir.dt.float32r
    bf = mybir.dt.bfloat16
    nc = _mk_bass()

    d8 = mybir.dt.float8e4
    i32 = mybir.dt.int32
    DR = mybir.MatmulPerfMode.DoubleRow
    xnt = nc.dram_tensor("xnt", [D, N], d8, kind="ExternalInput").ap()
    # qkv weights DR-packed+concatenated: [p, 3 proj, pair, plane, m], *64
    wqkv = nc.dram_tensor("wqkv", [128, 3 * D], d8, kind="ExternalInput").ap()
    wo = nc.dram_tensor("wo", [128, D], bf, kind="ExternalInput").ap()
    cos2 = nc.dram_tensor("cos2", [128, N], bf, kind="ExternalInput").ap()
    sin2 = nc.dram_tensor("sin2", [128, N], bf, kind="ExternalInput").ap()
    maskv = nc.dram_tensor("maskv", [128, 2], dt, kind="ExternalInput").ap()
    onesv = nc.dram_tensor("onesv", [1, 64], dtr, kind="ExternalInput").ap()
    ones64 = nc.dram_tensor("ones64", [128, 64], bf, kind="ExternalInput").ap()
    ident = nc.dram_tensor("ident", [128, 128], bf, kind="ExternalInput").ap()
    partial = nc.dram_tensor("partial", [N, D], bf, kind="ExternalOutput").ap()

    xnt_r = xnt.rearrange("(g two p) t -> p g two t", two=2, p=128)  # DR pairs
    # paired output view: 16 x 128p x (2 token-tiles x 1024)
    partial_r2 = partial.rearrange("(g two p) d -> g p two d", p=128, two=2)

    KC = D // 128      # 8 contraction chunks
    TC = N // 512      # 8 token chunks of 512
    NTT = N // 128     # 32 token tiles of 128

    with tile.TileContext(nc) as tc:
        with (
            tc.tile_pool(name="persist", bufs=1) as pp,
            tc.tile_pool(name="waug", bufs=1) as vpool,
        ):
            # persistent sbuf tensors
            qT = pp.tile([128, N], bf, tag="qT")
            kTz = [pp.tile([128, N], bf, tag=f"kTz{h}", name=f"kTz{h}") for h in range(HPC)]
            cos_t = pp.tile([128, N], bf, tag="cos")
            sin_t = pp.tile([128, N], bf, tag="sin")
            mask_t = pp.tile([128, 2], dt, tag="mask")
            aout = pp.tile([128, N], bf, tag="aout")   # normalized attn out^T
            ones_t = pp.tile([1, 64], dtr, tag="ones")
            ident_t = pp.tile([128, 128], bf, tag="ident")
            wqkv_b = pp.tile([128, 3 * D], d8, tag="wqkvb")
            wq_b, wk_b, wv_b = (wqkv_b[:, i * D:(i + 1) * D] for i in range(3))
            wo_b = pp.tile([128, D], bf, tag="wob")
            # all 32 v tiles live in one (128 x 32*130) tensor
            # v layout for DoubleRow: per (pair, head): two CONTIGUOUS
            # 128-wide planes [v 64 | one | pad 63][v 64 | one | pad 63]
            v_big = vpool.tile([128, NTT * 512], d8, tag="vbig")
            v2r = v_big.rearrange("p (pp h pl c) -> p pp h pl c",
                                  h=2, pl=2, c=128)
            # DR weight views: [p, pair, plane, m]
            wq_t = wq_b.rearrange("p (pr two m) -> p pr two m", pr=4, two=2)
            wk_t = wk_b.rearrange("p (pr two m) -> p pr two m", pr=4, two=2)
            wv_t = wv_b.rearrange("p (pr two m) -> p pr two m", pr=4, two=2)
            wo_t = [wo_b[:, ts(i, 512)] for i in range(2)]


            nc.sync.dma_start(out=wqkv_b, in_=wqkv)

            def _late_setup(step):
                # interleaved with the xnt stream: cos/sin arrive in 1024-col
                # chunks just before the rope that needs them; the rest slots
                # into spare DMA slots mid-phase
                q = ts(step, 1024)
                nc.sync.dma_start(out=cos_t[:, q], in_=cos2[:, q])
                nc.sync.dma_start(out=sin_t[:, q], in_=sin2[:, q])
                if step == 0:
                    nc.sync.dma_start(out=ident_t, in_=ident)
                    nc.sync.dma_start(out=mask_t, in_=maskv)
                    nc.sync.dma_start(out=ones_t, in_=onesv)
                elif step == 1:
                    # zero the pads, then one strided memset fills every
                    # ones column (block offset 64, stride 128)
                    nc.gpsimd.memset(
                        v_big.rearrange("p (blk c) -> p blk c", c=128)[:, :, 65:128],
                        0.0)
                    nc.gpsimd.memset(v_big[:, 64::128], 1.0)
                elif step == 2:
                    nc.sync.dma_start(out=wo_b, in_=wo)

            # -------- phase 1: qkv projections + rope + head masks + v_aug --------
            with (
                tc.tile_pool(name="xnstream", bufs=28) as xp,
                tc.tile_pool(name="kraw", bufs=3) as kr_,
                tc.tile_pool(name="vtmp", bufs=3) as vt_,
                tc.tile_pool(name="ropetmp", bufs=3) as rp,
                tc.tile_pool(name="ps_a1", bufs=7, space="PSUM") as ps1,
                tc.tile_pool(name="ps_tr", bufs=1, space="PSUM") as pstr_,
            ):
                eng_a = nc.gpsimd if rope_gpsimd else nc.vector
                for tcix in range(TC):
                    sl = ts(tcix, 512)
                    pq = ps1.tile([128, 512], dt, tag="pqkv", name=f"pq{tcix}")
                    pk = ps1.tile([128, 512], dt, tag="pqkv", name=f"pk{tcix}")
                    pv = ps1.tile([128, 512], dt, tag="pqkv", name=f"pv{tcix}")
                    # one fp8 DMA brings all contraction pairs for this chunk
                    xt = xp.tile([128, 4096], d8, tag="xn")
                    xt_r = xt.rearrange("p (g two t) -> p g two t", g=4, two=2)
                    if tcix == 0:
                        # split so the first pair-group lands sooner
                        nc.sync.dma_start(out=xt_r[:, 0:1], in_=xnt_r[:, 0:1, :, sl])
                        nc.sync.dma_start(out=xt_r[:, 1:4], in_=xnt_r[:, 1:4, :, sl])
                    else:
                        nc.sync.dma_start(out=xt_r, in_=xnt_r[:, :, :, sl])
                    for pr in range(4):
                        st, sp = pr == 0, pr == 3
                        rhs = xt_r[:, pr]
                        nc.tensor.matmul(pq, wq_t[:, pr], rhs,
                                         start=st, stop=sp, perf_mode=DR)
                        nc.tensor.matmul(pk, wk_t[:, pr], rhs,
                                         start=st, stop=sp, perf_mode=DR)
                        nc.tensor.matmul(pv, wv_t[:, pr], rhs,
                                         start=st, stop=sp, perf_mode=DR)
                    if tcix % 2 == 0:
                        _late_setup(tcix // 2)
                    # ---- v: transpose (128d x 512tok) into 4 token tiles ----
                    # (the 1/64 undoes the fp8 weight scaling)
                    vts = vt_.tile([128, 512], bf, tag="vts")
                    nc.scalar.mul(vts, pv, 1.0 / 64.0)
                    for i in range(4):
                        tt = tcix * 4 + i
                        ptr = pstr_.tile([128, 128], bf, tag="ptr")
                        nc.tensor.transpose(ptr, vts[:, ts(i, 128)], ident_t)
                        nc.scalar.copy(
                            v2r[:, tt // 2, :, tt % 2, 0:64],
                            ptr.rearrange("p (two c) -> p two c", two=2))
                    # ---- q/k: copy out of psum, rope in place, mask k ----
                    kraw = kr_.tile([128, 512], bf, tag="kraw")
                    nc.scalar.copy(qT[:, sl], pq)
                    nc.scalar.copy(kraw, pk)
                    if upto >= 2:
                        for srct, dest in ((qT[:, sl], None), (kraw, None)):
                            t0 = srct[0:64, :]
                            t1 = srct[64:128, :]
                            a = rp.tile([64, 512], bf, tag="ra")
                            b = rp.tile([64, 512], bf, tag="rb")
                            c = rp.tile([64, 512], bf, tag="rc")
                            d_ = rp.tile([64, 512], bf, tag="rd")
                            nc.vector.tensor_mul(a, t0, cos_t[0:64, sl])
                            nc.vector.tensor_mul(b, t1, sin_t[64:128, sl])
                            nc.vector.tensor_mul(c, t0, sin_t[0:64, sl])
                            nc.vector.tensor_mul(d_, t1, cos_t[64:128, sl])
                            nc.vector.tensor_sub(t0, a, b)
                            nc.vector.tensor_add(t1, c, d_)
                        for h in range(HPC):
                            eng_a.tensor_scalar_mul(
                                kTz[h][:, sl], kraw, mask_t[:, h:h + 1]
                            )

            # ------- phase 3: scores/softmax/av + wo, one HEAD at a time -------
            # Sequential heads keep only ONE av accumulator (2 banks) live,
            # so the sc ring gets 3 slots (6 banks): slot reuse distance 3 kc
            # finally exceeds the ~1.9us scores->exp chain, hiding it.
            with (
                tc.tile_pool(name="ps_sc", bufs=3, space="PSUM") as pssc,
                tc.tile_pool(name="ps_av", bufs=1, space="PSUM") as psav,
                tc.tile_pool(name="attn", bufs=6) as ap_,
                tc.tile_pool(name="norm", bufs=2) as np_,
                tc.tile_pool(name="oout", bufs=4) as op_,
            ):
                pending_norm = [None]

                def _flush_norm():
                    # the deferred {bc, bcs, aout} chain of the PRIOR head,
                    # emitted a kc into the next one so the PE queue never
                    # head-blocks on the DVE reciprocal
                    if pending_norm[0] is None:
                        return
                    pav, pq0, ph, prden = pending_norm[0]
                    pending_norm[0] = None
                    bc = pssc.tile([64, 1024], dt, tag="sc", name=f"bc{ph}")
                    for j in range(2):
                        nc.tensor.matmul(bc[:, ts(j, 512)], ones_t,
                                         prden[:, ts(j, 512)],
                                         start=True, stop=True)
                    bcs = np_.tile([64, 1024], bf, tag="bcs", name=f"bcs{ph}")
                    nc.scalar.copy(bcs, bc)
                    nc.vector.tensor_mul(
                        aout[ph * 64:(ph + 1) * 64, pq0:pq0 + 1024],
                        pav[0:64, :], bcs,
                    )

                ecnt = [0]
                for b in range(B) if upto >= 3 else ():
                    for qh in range(2):  # two 1024-wide query halves
                        q0 = b * S + qh * 1024
                        qs = qT[:, q0:q0 + 1024]
                        for h in range(HPC):
                            avh = [None]

                            def _av_mms(pp_, atp):
                                # fp8 DoubleRow over a kc PAIR: planes are
                                # consecutive v_aug tiles (stride 130), taken
                                # 128 wide so the ISA accepts the stationary;
                                # out rows 0:64 = v, row 64 = ones-denom,
                                # rows 65:127 = garbage (never read)
                                vv = v2r[:, (b * 16) // 2 + pp_, h]
                                atr = atp.rearrange("p (two c) -> p two c",
                                                    two=2)
                                for j in range(2):
                                    nc.tensor.matmul(
                                        avh[0][:, ts(j, 512)], vv,
                                        atr[:, :, ts(j, 512)],
                                        start=(pp_ == 0), stop=(pp_ == 7),
                                        perf_mode=DR,
                                    )

                            prevs = []
                            atp = None
                            for kc in range(16):
                                if kc == 1:
                                    # prior head's norm, then this head's av
                                    # accumulator (WAR captured by the alloc)
                                    _flush_norm()
                                    avh[0] = psav.tile([128, 1024], dt,
                                                       tag="av", name=f"av{h}")
                                sc = pssc.tile([128, 1024], dt, tag="sc",
                                               name=f"sc{kc}_{h}")
                                kz = kTz[h][:, b * S + kc * 128:
                                            b * S + kc * 128 + 128]
                                for j in range(2):
                                    nc.tensor.matmul(
                                        sc[:, ts(j, 512)], kz,
                                        qs[:, ts(j, 512)],
                                        start=True, stop=True)
                                if kc % 2 == 0:
                                    atp = ap_.tile([128, 2048], d8, tag="at",
                                                   name=f"atp{kc}_{h}")
                                at = atp[:, (kc % 2) * 1024:
                                         (kc % 2) * 1024 + 1024]
                                ecnt[0] += 1
                                if ecnt[0] % 3 == 0:
                                    # ~1/3 of exps on idle DVE via
                                    # Schraudolph bit-trick (~1.5% err,
                                    # harmless: attn out ~1% of h and the
                                    # denominator uses the same values)
                                    ti = np_.tile([128, 1024], i32,
                                                  tag="etmp", name=f"et{kc}")
                                    nc.vector.tensor_scalar(
                                        ti, sc, 12102203.0, 1064866805.0,
                                        mybir.AluOpType.mult,
                                        mybir.AluOpType.add)
                                    nc.vector.tensor_copy(at, ti.bitcast(dt))
                                else:
                                    nc.scalar.activation(
                                        at, sc,
                                        mybir.ActivationFunctionType.Exp,
                                    )
                                # av pair-matmuls run one pair behind the
                                # stream so deferred-norm chains never
                                # stall PE
                                if kc % 2 == 1:
                                    prevs.append((kc // 2, atp))
                                    if len(prevs) > 1:
                                        _av_mms(*prevs.pop(0))
                            for pv_ in prevs:
                                _av_mms(*pv_)
                            # reciprocal now; rest of the norm deferred
                            rden = np_.tile([1, 1024], dtr, tag="rden",
                                            name=f"rden{h}")
                            with nc.allow_low_precision("softmax denom recip"):
                                nc.vector.reciprocal(rden, avh[0][64:65, :])
                            pending_norm[0] = (avh[0], q0, h, rden)
                _flush_norm()
                # ---- wo partials, deferred: aout is complete, so the po
                # ring can borrow every psum slot without stalling a next
                # instance; evacuation split DVE/ACT ----
                po_tags = ["av", "sc", "sc", "sc"]
                for i in range(NTT if upto >= 3 else 0):
                    tt = i
                    if i % 2 == 0:
                        ot = op_.tile([128, 2048], bf, tag="ot")
                    po = (psav if i % 4 == 0 else pssc).tile(
                        [128, 1024], dt, tag=po_tags[i % 4], name=f"po{tt}")
                    for j in range(2):
                        nc.tensor.matmul(
                            po[:, ts(j, 512)], aout[:, ts(tt, 128)], wo_t[j],
                            start=True, stop=True,
                        )
                    osl = ot[:, (i % 2) * 1024:(i % 2) * 1024 + 1024]
                    if i % 2 == 0:
                        nc.vector.tensor_copy(osl, po)
                    else:
                        nc.scalar.copy(osl, po)
                        nc.sync.dma_start(out=partial_r2[tt // 2], in_=ot)

    nc.compile()
    return nc


def _build_moe(cap=CAP):
    """Launch B: routed expert (cap tokens) + shared expert (SHARE tokens).

    fp8e4m3 DoubleRow matmuls throughout. Host scales: w1*64, w3*16, w2*64;
    silu uses scale=1/64 so h13 = silu(z1) * (z3*16); the final per-token
    scale vector folds in 1/(16*64).
    """
    import concourse.mybir as mybir
    import concourse.tile as tile
    from concourse.bass import ts

    DR = mybir.MatmulPerfMode.DoubleRow
    TPAD = cap + SHARE
    dt = mybir.dt.float32
    bf = mybir.dt.bfloat16
    d8 = mybir.dt.float8e4
    nc = _mk_bass()

    xft = nc.dram_tensor("xft", [D, TPAD], d8, kind="ExternalInput").ap()
    # w13 packed per hc: [p, hc, pair, plane, m] (value already *64 / *16, fp8)
    w1e = nc.dram_tensor("w1e", [128, HC_ * D], d8, kind="ExternalInput").ap()
    w3e = nc.dram_tensor("w3e", [128, HC_ * D], d8, kind="ExternalInput").ap()
    w1s = nc.dram_tensor("w1s", [128, HC_ * D], d8, kind="ExternalInput").ap()
    w3s = nc.dram_tensor("w3s", [128, HC_ * D], d8, kind="ExternalInput").ap()
    # w2 packed: [p, pair, plane, d] (value *64, fp8)
    w2e = nc.dram_tensor("w2e", [128, 8 * D], d8, kind="ExternalInput").ap()
    w2s = nc.dram_tensor("w2s", [128, 8 * D], d8, kind="ExternalInput").ap()
    scale = nc.dram_tensor("scale", [TPAD, 1], dt, kind="ExternalInput").ap()
    out = nc.dram_tensor("out", [TPAD, D], bf, kind="ExternalOutput").ap()

    NTT = TPAD // 128
    NTT_E = cap // 128
    xft_r = xft.rearrange("(kc p) t -> p kc t", p=128)
    out_r = out.rearrange("(tt p) d -> tt p d", p=128)
    out_r2 = out[0:(NTT // 2) * 256, :].rearrange(
        "(g two p) d -> g p two d", p=128, two=2
    )
    scale_p = scale.rearrange("(tt p) o -> p (tt o)", p=128)

    # routed-token column chunks
    chunks_e = [(c0, min(512, cap - c0)) for c0 in range(0, cap, 512)]
    chunks_s = [(cap, 512)]

    with tile.TileContext(nc) as tc:
        with (
            tc.tile_pool(name="xf", bufs=1) as xfp,
            tc.tile_pool(name="h13", bufs=1) as hp,
            tc.tile_pool(name="w2blk", bufs=1) as w2p,
            tc.tile_pool(name="scl", bufs=1) as scp,
        ):
            xf8 = xfp.tile([128, KC * TPAD], d8, tag="xf8")
            xf8_r = xf8.rearrange("p (kc t) -> p kc t", kc=KC)
            # h13 pair tiles: tile i holds hm-block 2i (cols 0:TPAD) and
            # 2i+1 (cols TPAD:2*TPAD), fp8
            h13p = [hp.tile([128, 2 * TPAD], d8, tag=f"h13p{i}", name=f"h13p{i}")
                    for i in range(4)]
            h13r = [t.rearrange("p (two c) -> p two c", two=2) for t in h13p]
            sct_b = scp.tile([128, NTT], dt, tag="sctb")
            w2bigs = {nm: w2p.tile([128, 8 * D], d8, tag="w2big", name=f"w2big{nm}")
                      for nm in ("e", "s")}
            w2r = {nm: w2bigs[nm].rearrange("p (pr two d) -> p pr two d", pr=4, two=2)
                   for nm in ("e", "s")}

            # ------------ phase 1: h13 = silu(x@w1) * (x@w3) ------------
            with (
                tc.tile_pool(name="wblk", bufs=16) as wp,
                tc.tile_pool(name="silu", bufs=3) as sp_,
                tc.tile_pool(name="ps", bufs=8, space="PSUM") as psr,
                tc.tile_pool(name="oout", bufs=8) as op_,
            ):
                wtiles = {}

                def _load_w13(hc):
                    for sfx, w1d, w3d in (("e", w1e, w3e), ("s", w1s, w3s)):
                        t1b = wp.tile([128, D], d8, tag="w1b", name=f"w1b{hc}{sfx}")
                        t3b = wp.tile([128, D], d8, tag="w3b", name=f"w3b{hc}{sfx}")
                        nc.sync.dma_start(out=t1b, in_=w1d[:, ts(hc, D)])
                        if hc == 0 and sfx == "e":
                            # first xft slab on the Pool SWDGE queue: issues
                            # in parallel with the SP/HWDGE weight stream
                            nc.gpsimd.dma_start(
                                out=xf8_r[:, :, 0:512], in_=xft_r[:, :, 0:512]
                            )
                        nc.sync.dma_start(out=t3b, in_=w3d[:, ts(hc, D)])
                        if hc == 0 and sfx == "e":
                            nc.gpsimd.dma_start(
                                out=xf8_r[:, :, 512:1024], in_=xft_r[:, :, 512:1024]
                            )
                        wtiles[(hc, sfx)] = (t1b, t3b)

                # Front-load every input DMA: no WAR hazards anywhere (all
                # weight tiles stay resident), so SP issues the whole stream
                # back-to-back and transfers order by need.
                _load_w13(0)
                nc.gpsimd.dma_start(out=xf8_r[:, :, 1024:TPAD],
                                    in_=xft_r[:, :, 1024:TPAD])
                nc.gpsimd.dma_start(out=sct_b, in_=scale_p)
                _load_w13(1)
                for i in range(2):
                    nc.gpsimd.dma_start(out=w2bigs["e"][:, i * 4 * D:(i + 1) * 4 * D],
                                        in_=w2e[:, i * 4 * D:(i + 1) * 4 * D])
                _load_w13(2)
                for i in range(2):
                    nc.gpsimd.dma_start(out=w2bigs["s"][:, i * 4 * D:(i + 1) * 4 * D],
                                        in_=w2s[:, i * 4 * D:(i + 1) * 4 * D])
                for hc in range(3, HC_):
                    _load_w13(hc)

                for hc in range(HC_):
                    for (w1d, w3d, chunks, sfx) in (
                        (w1e, w3e, chunks_e, "e"),
                        (w1s, w3s, chunks_s, "s"),
                    ):
                        t1b, t3b = wtiles[(hc, sfx)]
                        w1b = t1b.rearrange("p (pr two m) -> p pr two m", pr=4, two=2)
                        w3b = t3b.rearrange("p (pr two m) -> p pr two m", pr=4, two=2)
                        hdst = h13p[hc // 2]
                        hoff = (hc % 2) * TPAD
                        for (c0, cw) in chunks:
                            p1 = psr.tile([128, 512], dt, tag="ps", name=f"p1_{hc}{sfx}{c0}")
                            p3 = psr.tile([128, 512], dt, tag="ps", name=f"p3_{hc}{sfx}{c0}")
                            for pr in range(4):
                                st, sp = pr == 0, pr == 3
                                rhs = xf8_r[:, 2 * pr:2 * pr + 2, c0:c0 + cw]
                                nc.tensor.matmul(p1[:, 0:cw], w1b[:, pr], rhs,
                                                 start=st, stop=sp, perf_mode=DR)
                                nc.tensor.matmul(p3[:, 0:cw], w3b[:, pr], rhs,
                                                 start=st, stop=sp, perf_mode=DR)
                            sg = sp_.tile([128, 512], bf, tag="sg")
                            nc.scalar.activation(
                                sg[:, 0:cw], p1[:, 0:cw],
                                mybir.ActivationFunctionType.Silu,
                                scale=1.0 / 64.0,
                            )
                            nc.vector.tensor_mul(
                                hdst[:, hoff + c0:hoff + c0 + cw],
                                sg[:, 0:cw], p3[:, 0:cw],
                            )

                # ---------- phase 2: out = (h13 @ w2) * scale ----------
                for (nm, tt0, tt1) in (("e", 0, NTT_E), ("s", NTT_E, NTT)):
                    w2b = w2r[nm]
                    for tt in range(tt0, tt1):
                        ot = op_.tile([128, 1024], bf, tag="ot")
                        sct_c = sct_b[:, tt:tt + 1]
                        pos = [psr.tile([128, 512], dt, tag="ps", name=f"po{nm}{tt}_{j}")
                               for j in range(2)]
                        for pr in range(4):
                            st, sp = pr == 0, pr == 3
                            lhs = h13r[pr][:, :, ts(tt, 128)]
                            for j in range(2):
                                nc.tensor.matmul(
                                    pos[j], lhs, w2b[:, pr, :, ts(j, 512)],
                                    start=st, stop=sp, perf_mode=DR,
                                )
                        # split the evacuation across DVE+ACT so psum slots
                        # free faster than the matmul cadence
                        nc.vector.tensor_scalar_mul(ot[:, 0:512], pos[0], sct_c)
                        nc.scalar.mul(ot[:, 512:1024], pos[1], sct_c)
                        nc.sync.dma_start(out=out_r[tt], in_=ot)

    nc.compile()
    return nc


def _programs():
    if "A" not in _CACHE:
        _CACHE["A"] = _build_attn()
    if "Bp" not in _CACHE:
        _CACHE["Bp"] = _build_moe()
    return _CACHE["A"], _CACHE["Bp"]


def _run(nc, in_maps, trace=False):
    from concourse.bass_utils import run_bass_kernel_spmd
    return run_bass_kernel_spmd(nc, in_maps, list(range(NCORES)), trace=trace)


# --------------------------------------------------------------------------
# host-side orchestration
# --------------------------------------------------------------------------

def _rmsnorm(x, w):
    return x * (1.0 / np.sqrt((x * x).mean(-1, keepdims=True) + EPS)) * w


_PERM = np.concatenate([
    np.arange(0, 64, 2), 64 + np.arange(0, 64, 2),
    np.arange(1, 64, 2), 64 + np.arange(1, 64, 2),
])  # within a core's 128-col block: [h0 even, h1 even, h0 odd, h1 odd]

_MASKV = np.zeros((128, 2), FP32)
_MASKV[0:32, 0] = 1.0
_MASKV[64:96, 0] = 1.0
_MASKV[32:64, 1] = 1.0
_MASKV[96:128, 1] = 1.0


def prep_attn_inputs(x, freqs_cos, freqs_sin, att_norm_w, wq, wk, wv, wo):
    import ml_dtypes
    BF = ml_dtypes.bfloat16
    F8 = ml_dtypes.float8_e4m3fn
    xn = _rmsnorm(x.reshape(N, D), att_norm_w)
    xnt = np.ascontiguousarray(xn.T).astype(F8)
    cosT = np.ascontiguousarray(freqs_cos.T)    # (32, S)
    sinT = np.ascontiguousarray(freqs_sin.T)
    cos2 = np.ascontiguousarray(np.tile(np.hstack([cosT] * B), (4, 1))).astype(BF)
    sin2 = np.ascontiguousarray(np.tile(np.hstack([sinT] * B), (4, 1))).astype(BF)
    wk_s = (wk * (1.0 / np.sqrt(HD))).astype(FP32)

    def pack8(w, s):
        # (1024, 128) -> DR layout [p, pair, plane, m] = [128, 1024] fp8
        v = (np.asarray(w, FP32) * s).reshape(4, 2, 128, 128)
        return np.ascontiguousarray(
            v.transpose(2, 0, 1, 3).reshape(128, 1024)
        ).astype(F8)

    in_maps = []
    for c in range(NCORES):
        blk = slice(c * 128, (c + 1) * 128)
        in_maps.append({
            "xnt": xnt,
            "wqkv": np.concatenate([
                pack8(wq[:, blk][:, _PERM], 64.0),
                pack8(wk_s[:, blk][:, _PERM], 64.0),
                pack8(wv[:, blk], 64.0),
            ], axis=1),
            "wo": np.ascontiguousarray(wo[blk, :]).astype(BF),
            "cos2": cos2,
            "sin2": sin2,
            "maskv": _MASKV * (1.0 / 4096.0),
            "onesv": np.ones((1, 64), FP32),
            "ones64": np.ones((128, 64), BF),
            "ident": np.eye(128, dtype=FP32).astype(BF),
        })
    return in_maps


def _pack_w13(w, s):
    """(D, HM) -> [128, HC_*D] fp8: [p, hc, pair, plane, m] = w[pr*256+pl*128+p, hc*128+m]*s"""
    import ml_dtypes
    v = (np.asarray(w, FP32) * s).reshape(4, 2, 128, HC_, 128)
    return np.ascontiguousarray(
        v.transpose(2, 3, 0, 1, 4).reshape(128, HC_ * D)
    ).astype(ml_dtypes.float8_e4m3fn)


def _pack_w2(w2, s):
    """(HM, D) -> [128, 8*D] fp8: [p, pair, plane, d] = w2[pr*256+pl*128+p, d]*s"""
    import ml_dtypes
    v = (np.asarray(w2, FP32) * s).reshape(4, 2, 128, D)
    return np.ascontiguousarray(
        v.transpose(2, 0, 1, 3).reshape(128, 8 * D)
    ).astype(ml_dtypes.float8_e4m3fn)


def route(xf, gate_w):
    g = xf @ gate_w.T
    g = g - g.max(-1, keepdims=True)
    p = np.exp(g)
    p /= p.sum(-1, keepdims=True)
    idx = np.argsort(-p, axis=1, kind="stable")[:, :TOPK]      # (N, 2)
    vals = np.take_along_axis(p, idx, axis=1)
    w = vals / (vals.sum(-1, keepdims=True) + 1e-9)
    experts = []
    for e in range(E):
        m = idx == e
        tok = np.nonzero(m.any(1))[0]
        wt = (w * m).sum(1)[tok]
        experts.append((tok, wt.astype(FP32)))
    return experts


def kernel(**inputs):
    ins = {k: np.ascontiguousarray(np.asarray(v)) for k, v in inputs.items()}
    x = ins["x"].astype(FP32, copy=False)
    nc_a, _ = _programs()

    # ----- launch A: attention -----
    in_maps = prep_attn_inputs(
        x, ins["freqs_cos"], ins["freqs_sin"], ins["att_norm_w"],
        ins["wq"], ins["wk"], ins["wv"], ins["wo"],
    )
    res_a = _run(nc_a, in_maps, trace=_CACHE.get("trace", False))
    _CACHE["res_a"] = res_a

    h = x.reshape(N, D).copy()
    for c in range(NCORES):
        h += res_a.results[c]["partial"].astype(FP32)

    # ----- host routing -----
    xf = _rmsnorm(h, ins["ffn_norm_w"])
    experts = route(xf, ins["gate_w"])

    # capacity: default CAP covers the reference routing with margin; fall
    # back to a one-off rebuild if some expert ever exceeds it
    max_ct = max(len(t) for t, _ in experts)
    cap = CAP if max_ct <= CAP else ((max_ct + 127) // 128) * 128
    key = f"Bp{cap}"
    if key not in _CACHE:
        _CACHE[key] = _CACHE.get("Bp") if cap == CAP else _build_moe(cap)
        if _CACHE[key] is None:
            _CACHE[key] = _build_moe(cap)
    nc_b = _CACHE[key]
    tpad = cap + SHARE

    import ml_dtypes
    F8 = ml_dtypes.float8_e4m3fn
    xf8T = np.ascontiguousarray(xf.T).astype(F8)     # (D, N) fp8

    w1s8 = _pack_w13(ins["sw1"], SW1)
    w3s8 = _pack_w13(ins["sw3"], SW3)
    w2s8 = _pack_w2(ins["sw2"], SW2)
    in_maps_b = []
    for c in range(NCORES):
        tok, wt = experts[c]
        ct = len(tok)
        xft = np.zeros((D, tpad), F8)
        xft[:, :ct] = xf8T[:, tok]
        xft[:, cap:] = xf8T[:, c * SHARE:(c + 1) * SHARE]
        sc = np.zeros((tpad, 1), FP32)
        sc[:ct, 0] = wt / (SW3 * SW2)
        sc[cap:, 0] = 1.0 / (SW3 * SW2)
        in_maps_b.append({
            "xft": xft,
            "w1e": _pack_w13(ins["ew1"][c], SW1),
            "w3e": _pack_w13(ins["ew3"][c], SW3),
            "w2e": _pack_w2(ins["ew2"][c], SW2),
            "w1s": w1s8, "w3s": w3s8, "w2s": w2s8,
            "scale": sc,
        })
    res_b = _run(nc_b, in_maps_b, trace=_CACHE.get("trace", False))
    _CACHE["res_b"] = res_b

    # ----- combine -----
    y = h.copy()
    for c in range(NCORES):
        o = res_b.results[c]["out"].astype(FP32)
        tok, _ = experts[c]
        ct = len(tok)
        y[tok] += o[:ct]
        y[c * SHARE:(c + 1) * SHARE] += o[cap:]
    return y.reshape(B, S, D).astype(ins["x"].dtype, copy=False)

